# revision 1
# baseline (speedup 1.0000x reference)
"""CrossBlock kernel for Trainium2, 8-core SPMD.

Reference computation (B=2, N=4096, D=256, H=4):
  shared-qk cross-attention between two streams x0, x1 + per-stream FFN
  with layernorm and exact gelu.

Sharding: row-parallel. Core c in 0..7 handles batch b=c//4 and query-row
block r=c%4 (1024 rows) of BOTH streams. Each core holds the full keys /
values of its batch (computed on-core from the full x[b]) and produces
out0[b, rows], out1[b, rows]. No collectives.

Everything on-device is feature-major ([features, rows]); the host
pre-transposes inputs and post-transposes outputs, so the device never
transposes anything.
"""

import numpy as np
import ml_dtypes

import concourse.bass as bass
import concourse.mybir as mybir
import concourse.tile as tile
from concourse import bacc
from concourse.alu_op_type import AluOpType
from concourse.bass_utils import run_bass_kernel_spmd

BF16 = mybir.dt.bfloat16
F32 = mybir.dt.float32
NPBF = ml_dtypes.bfloat16
AF = mybir.ActivationFunctionType

B, N, D, H = 2, 4096, 256, 4
DH = D // H  # 64
D2 = 2 * D   # 512
SCALE = DH ** -0.25  # applied to both q and k sides (folded into Wqk on host)


def build_program(nq=N // 4, nk=N, act_sim=False, reps=1):
    """Build the single-core program (identical across the 8 cores)."""
    gelu_f = AF.Sigmoid if act_sim else AF.Gelu
    qb_n = nq // 512    # query 512-blocks
    nb_n = nk // 512    # key 512-blocks
    kb_n = nk // 128    # key 128-blocks

    nc = bacc.Bacc("TRN2", target_bir_lowering=False, debug=False)

    def din(name, shape, dt):
        return nc.dram_tensor(name, shape, dt, kind="ExternalInput").ap()

    def dout(name, shape, dt):
        return nc.dram_tensor(name, shape, dt, kind="ExternalOutput").ap()

    xk = [din(f"xk{s}", [D, nk], BF16) for s in range(2)]
    xq = [din(f"xq{s}", [D, nq], BF16) for s in range(2)]
    xqf = [din(f"xqf{s}", [D, nq], F32) for s in range(2)]
    wqk_d = din("wqk", [D, D], BF16)        # pre-scaled by SCALE on host
    wv_d = din("wv", [D, D], BF16)
    wo_d = din("wo", [D, D], BF16)
    w1_d = din("w1", [D2, D2], BF16)
    w2_d = din("w2", [D2, D], BF16)
    bqk_d = din("bqk", [128, 2], F32)       # pre-scaled, [p, m] = b[m*128+p]
    bv_d = din("bv", [1, D], BF16)
    bo_d = din("bo", [128, 2], F32)
    b1_d = din("b1", [128, 4], F32)
    g1_d = din("g1", [128, 4], F32)         # ln_g
    lb_d = din("lb", [128, 4], F32)         # ln_b
    b2_d = din("b2", [128, 2], F32)
    o_dram = [dout(f"o{s}", [D, nq], F32) for s in range(2)]

    import contextlib
    with tile.TileContext(nc) as tc, contextlib.ExitStack() as ctx:
        P = ctx.enter_context  # noqa: E741

        persist = P(tc.tile_pool(name="persist", bufs=1))
        xkp = P(tc.tile_pool(name="xkp", bufs=4))
        ptp = P(tc.tile_pool(name="ptp", bufs=5))
        repp = P(tc.tile_pool(name="repp", bufs=3))
        scr = P(tc.tile_pool(name="scr", bufs=3))
        arena = P(tc.tile_pool(name="arena", bufs=1))
        ps_sim = P(tc.tile_pool(name="ps_sim", bufs=2, space="PSUM"))
        ps_pv = P(tc.tile_pool(name="ps_pv", bufs=2, space="PSUM"))
        ps_oth = P(tc.tile_pool(name="ps_oth", bufs=2, space="PSUM"))
        dram = P(tc.tile_pool(name="dram", bufs=2, space="DRAM"))

        # ---- constants (wqk/wv first: they gate the first projections) ----
        wqk = [persist.tile([128, D], BF16, name=f"wqk{k}", tag=f"wqk{k}") for k in range(2)]
        wv = [persist.tile([128, D], BF16, name=f"wv{k}", tag=f"wv{k}") for k in range(2)]
        wo = persist.tile([64, H, D], BF16, name="wo_r", tag="wo_r")
        w1 = [persist.tile([128, D2], BF16, name=f"w1{k}", tag=f"w1{k}") for k in range(4)]
        w2 = [persist.tile([128, D], BF16, name=f"w2{k}", tag=f"w2{k}") for k in range(4)]
        for k in range(2):
            nc.sync.dma_start(out=wqk[k], in_=wqk_d[k * 128:(k + 1) * 128, :])
            nc.sync.dma_start(out=wv[k], in_=wv_d[k * 128:(k + 1) * 128, :])
        bqk = persist.tile([128, 2], F32, name="bqk", tag="bqk")
        bo = persist.tile([128, 2], F32, name="bo", tag="bo")
        b1 = persist.tile([128, 4], F32, name="b1", tag="b1")
        g1 = persist.tile([128, 4], F32, name="g1", tag="g1")
        lb = persist.tile([128, 4], F32, name="lb", tag="lb")
        b2 = persist.tile([128, 2], F32, name="b2", tag="b2")
        for t, d in [(bqk, bqk_d), (bo, bo_d), (b1, b1_d),
                     (g1, g1_d), (lb, lb_d), (b2, b2_d)]:
            nc.sync.dma_start(out=t, in_=d)
        bv_rep = persist.tile([128, H, DH], BF16, name="bv_rep", tag="bv_rep")
        nc.sync.dma_start(
            out=bv_rep,
            in_=bv_d.rearrange("o (h m) -> o h m", h=H).partition_broadcast(128))
        ones_c = persist.tile([128, 1], BF16, name="ones_c", tag="ones_c")   # stats lhsT
        nc.vector.memset(ones_c, 1.0)
        eps = persist.tile([1, 1], F32, name="eps", tag="eps")
        nc.vector.memset(eps, 1e-5)

        # ---- persistent activations ----
        xq_sb = [[persist.tile([128, nq], BF16, name=f"xq{s}{k}", tag=f"xq{s}{k}") for k in range(2)]
                 for s in range(2)]
        for s in range(2):
            for k in range(2):
                nc.sync.dma_start(out=xq_sb[s][k], in_=xq[s][k * 128:(k + 1) * 128, :])
        nc.sync.dma_start(out=wo, in_=wo_d.rearrange("(h p) m -> p h m", p=64))
        for k in range(4):
            nc.sync.dma_start(out=w1[k], in_=w1_d[k * 128:(k + 1) * 128, :])
            nc.sync.dma_start(out=w2[k], in_=w2_d[k * 128:(k + 1) * 128, :])

        QT = [[persist.tile([128, nq], BF16, name=f"QT{s}{hp}", tag=f"QT{s}{hp}") for hp in range(2)]
              for s in range(2)]
        KT = [[[persist.tile([128, 512], BF16, name=f"KT{s}{hp}{nb}", tag=f"KT{s}{hp}{nb}")
                for nb in range(nb_n)] for hp in range(2)] for s in range(2)]
        V = [[persist.tile([128, H, DH + 1], BF16, name=f"V{s}{kb}", tag=f"V{s}{kb}") for kb in range(kb_n)]
             for s in range(2)]
        M0raw = [persist.tile([64, H, nq], BF16, name=f"M0raw{d}", tag=f"M0raw{d}") for d in range(2)]
        HT = [[persist.tile([128, nq], BF16, name=f"HT{s}{m}", tag=f"HT{s}{m}") for m in range(2)]
              for s in range(2)]  # to_out outputs; ffn1 rhs tiles 2,3

        # ================= phase bodies =================
        sums_scr = [dram.tile([H, nq], F32, name=f"sums{d}", tag=f"sums{d}") for d in range(2)]
        recs_scr = [dram.tile([H, nq], F32, name=f"recs{d}", tag=f"recs{d}") for d in range(2)]

        def proj_q(s):
            for hp in range(2):
                for qb in range(qb_n):
                    ps = ps_oth.tile([128, 512], F32, name="oth", tag="oth")
                    nc.tensor.matmul(ps[:, 0:512], lhsT=wqk[0][:, hp * 128:(hp + 1) * 128],
                                     rhs=xq_sb[s][0][:, qb * 512:(qb + 1) * 512],
                                     start=True, stop=False)
                    nc.tensor.matmul(ps[:, 0:512], lhsT=wqk[1][:, hp * 128:(hp + 1) * 128],
                                     rhs=xq_sb[s][1][:, qb * 512:(qb + 1) * 512],
                                     start=False, stop=True)
                    nc.vector.tensor_scalar_add(
                        QT[s][hp][:, qb * 512:(qb + 1) * 512], ps[:, 0:512],
                        bqk[:, hp:hp + 1])

        def proj_kv(s):
            for nb in range(nb_n):
                xt0 = xkp.tile([128, 512], BF16, name="xt0", tag="xt0")
                xt1 = xkp.tile([128, 512], BF16, name="xt1", tag="xt1")
                nc.sync.dma_start(out=xt0, in_=xk[s][0:128, nb * 512:(nb + 1) * 512])
                nc.sync.dma_start(out=xt1, in_=xk[s][128:256, nb * 512:(nb + 1) * 512])
                for hp in range(2):
                    ps = ps_oth.tile([128, 512], F32, name="oth", tag="oth")
                    nc.tensor.matmul(ps[:, 0:512], lhsT=wqk[0][:, hp * 128:(hp + 1) * 128],
                                     rhs=xt0, start=True, stop=False)
                    nc.tensor.matmul(ps[:, 0:512], lhsT=wqk[1][:, hp * 128:(hp + 1) * 128],
                                     rhs=xt1, start=False, stop=True)
                    nc.vector.tensor_scalar_add(
                        KT[s][hp][nb], ps[:, 0:512], bqk[:, hp:hp + 1])
                for j in range(4):
                    kb = nb * 4 + j
                    ps = ps_oth.tile([128, 512], F32, name="oth", tag="oth")
                    nc.tensor.matmul(ps[:, 0:D], lhsT=xt0[:, j * 128:(j + 1) * 128],
                                     rhs=wv[0], start=True, stop=False)
                    nc.tensor.matmul(ps[:, 0:D], lhsT=xt1[:, j * 128:(j + 1) * 128],
                                     rhs=wv[1], start=False, stop=True)
                    nc.vector.tensor_tensor(
                        V[s][kb][:, :, 0:DH],
                        ps[:, 0:D].rearrange("p (h m) -> p h m", h=H),
                        bv_rep, op=AluOpType.add)
                    nc.vector.memset(V[s][kb][:, :, DH:DH + 1], 1.0)

        def finish_qb(di, qb):
            qsl = slice(qb * 512, (qb + 1) * 512)
            for m in range(2):
                ps = ps_oth.tile([128, 512], F32, name="oth", tag="oth")
                for h in range(H):
                    nc.tensor.matmul(
                        ps[:, 0:512],
                        lhsT=wo[:, h, m * 128:(m + 1) * 128],
                        rhs=M0raw[di][:, h, qsl],
                        start=(h == 0), stop=(h == H - 1))
                nc.vector.tensor_scalar_add(HT[di][m][:, qsl], ps[:, 0:512],
                                            bo[:, m:m + 1])
            ffn_qb(di, qb)

        pending = []

        def flush_pending():
            while pending:
                finish_qb(*pending.pop(0))

        def attention(di):
            q_s, k_s = di, 1 - di
            for qb in range(qb_n):
                for hp in range(2):
                    if hp == 1:
                        flush_pending()
                    pv = [ps_pv.tile([65, 512], F32, name="pv", tag="pv")
                          for _ in range(2)]
                    pv_defer = []
                    for kb in range(kb_n):
                        ps = ps_sim.tile([128, 1024], F32, name="sim", tag="sim")
                        kt = KT[k_s][hp][kb // 4]
                        ksl = slice((kb % 4) * 128, (kb % 4 + 1) * 128)
                        nc.tensor.matmul(
                            ps[:, 0:512],
                            lhsT=kt[0:64, ksl],
                            rhs=QT[q_s][hp][0:64, qb * 512:(qb + 1) * 512],
                            start=True, stop=True, tile_position=(0, 0))
                        nc.tensor.matmul(
                            ps[:, 512:1024],
                            lhsT=kt[64:128, ksl],
                            rhs=QT[q_s][hp][64:128, qb * 512:(qb + 1) * 512],
                            start=True, stop=True, tile_position=(64, 0))
                        pt = ptp.tile([128, 1024], BF16, name="pt", tag="pt")
                        nc.scalar.activation(out=pt, in_=ps, func=AF.Exp)
                        pv_defer.append((kb, pt))
                        if len(pv_defer) > 3:
                            dkb, dpt = pv_defer.pop(0)
                            for h01 in range(2):
                                nc.tensor.matmul(
                                    pv[h01],
                                    lhsT=V[k_s][dkb][:, 2 * hp + h01, :],
                                    rhs=dpt[:, h01 * 512:(h01 + 1) * 512],
                                    start=(dkb == 0), stop=(dkb == kb_n - 1))
                    while pv_defer:
                        dkb, dpt = pv_defer.pop(0)
                        for h01 in range(2):
                            nc.tensor.matmul(
                                pv[h01],
                                lhsT=V[k_s][dkb][:, 2 * hp + h01, :],
                                rhs=dpt[:, h01 * 512:(h01 + 1) * 512],
                                start=(dkb == 0), stop=(dkb == kb_n - 1))
                    # drain (qb, hp): raw copy + sums bounce + packed recip
                    qsl = slice(qb * 512, (qb + 1) * 512)
                    for h01 in range(2):
                        h = 2 * hp + h01
                        nc.vector.tensor_copy(M0raw[di][:, h, qsl], pv[h01][0:64, :])
                        sstage = scr.tile([65, 512], F32, name="sstage", tag="sstage")
                        nc.vector.tensor_copy(sstage[64:65, :], pv[h01][64:65, :])
                        nc.sync.dma_start(out=sums_scr[di][h:h + 1, qsl],
                                          in_=sstage[64:65, :])
                    rc_in = scr.tile([128, 8], F32, name="rc", tag="rc")
                    hp_rows = sums_scr[di][2 * hp:2 * hp + 2, qsl]
                    nc.sync.dma_start(out=rc_in, in_=hp_rows)
                    nc.vector.reciprocal(rc_in, rc_in)
                    rec_rows = recs_scr[di][2 * hp:2 * hp + 2, qsl]
                    nc.sync.dma_start(out=rec_rows, in_=rc_in)
                    for h01 in range(2):
                        h = 2 * hp + h01
                        rep = repp.tile([64, 512], F32, name="rep", tag="rep")
                        nc.sync.dma_start(
                            out=rep,
                            in_=recs_scr[di][h:h + 1, qsl].partition_broadcast(64))
                        nc.vector.tensor_tensor(M0raw[di][:, h, qsl],
                                                M0raw[di][:, h, qsl], rep,
                                                op=AluOpType.mult)
                # defer this block's to_out+ffn: flushed mid-attention of a
                # later block so PE keeps sim/PV work while the normalize
                # chain completes
                pending.append((di, qb))

        def ffn_qb(s, qb):
            """Column-local ffn for one 512-wide query block."""
            qsl = slice(qb * 512, (qb + 1) * 512)
            rhs1 = [xq_sb[s][0], xq_sb[s][1], HT[s][0], HT[s][1]]
            gpre = [arena.tile([128, nq], BF16, name=f"gpre{s}{m}", tag=f"gpre{s}{m}")
                    for m in range(4)]
            for m in range(4):
                ps = ps_oth.tile([128, 512], F32, name="oth", tag="oth")
                for k in range(4):
                    nc.tensor.matmul(ps[:, 0:512],
                                     lhsT=w1[k][:, m * 128:(m + 1) * 128],
                                     rhs=rhs1[k][:, qsl],
                                     start=(k == 0), stop=(k == 3))
                nc.vector.tensor_scalar_add(gpre[m][:, qsl], ps[:, 0:512],
                                            b1[:, m:m + 1])
            sq = [scr.tile([128, 512], BF16, name=f"sq{m}", tag="sq") for m in range(4)]
            for m in range(4):
                nc.vector.tensor_tensor(sq[m], gpre[m][:, qsl], gpre[m][:, qsl],
                                        op=AluOpType.mult)
            mu = arena.tile([1, nq], F32, name=f"mu{s}", tag=f"mu{s}")
            msq = arena.tile([1, nq], F32, name=f"msq{s}", tag=f"msq{s}")
            for dst, srcs, insl in [(mu, gpre, True), (msq, sq, False)]:
                ps = ps_oth.tile([128, 512], F32, name="oth", tag="oth")
                for k in range(4):
                    rhs_ = srcs[k][:, qsl] if insl else srcs[k]
                    nc.tensor.matmul(ps[0:1, 0:512], lhsT=ones_c, rhs=rhs_,
                                     start=(k == 0), stop=(k == 3))
                nc.vector.tensor_scalar(dst[:, qsl], ps[0:1, 0:512], 1.0 / D2, None,
                                        op0=AluOpType.mult)
            tmp = arena.tile([1, nq], F32, name=f"tmp{s}", tag=f"tmp{s}")
            nc.vector.tensor_tensor(tmp[:, qsl], mu[:, qsl], mu[:, qsl],
                                    op=AluOpType.mult)
            nc.vector.tensor_tensor(msq[:, qsl], msq[:, qsl], tmp[:, qsl],
                                    op=AluOpType.subtract)
            nc.scalar.activation(out=msq[:, qsl], in_=msq[:, qsl], func=AF.Sqrt,
                                 bias=eps)
            nc.vector.reciprocal(msq[:, qsl], msq[:, qsl])
            mu_scr = dram.tile([1, nq], F32, name=f"mu_scr{s}", tag=f"mu_scr{s}")
            rs_scr = dram.tile([1, nq], F32, name=f"rs_scr{s}", tag=f"rs_scr{s}")
            nc.sync.dma_start(out=mu_scr[:, qsl], in_=mu[:, qsl])
            nc.sync.dma_start(out=rs_scr[:, qsl], in_=msq[:, qsl])
            rep_mu = repp.tile([128, 512], F32, name="rep_mu", tag="rep")
            rep_rs = repp.tile([128, 512], F32, name="rep_rs", tag="rep")
            nc.sync.dma_start(out=rep_mu, in_=mu_scr[:, qsl].partition_broadcast(128))
            nc.sync.dma_start(out=rep_rs, in_=rs_scr[:, qsl].partition_broadcast(128))
            for m in range(4):
                nc.vector.tensor_tensor(gpre[m][:, qsl], gpre[m][:, qsl], rep_mu,
                                        op=AluOpType.subtract)
                nc.vector.tensor_tensor(gpre[m][:, qsl], gpre[m][:, qsl], rep_rs,
                                        op=AluOpType.mult)
                nc.scalar.activation(out=gpre[m][:, qsl], in_=gpre[m][:, qsl],
                                     func=gelu_f, bias=lb[:, m:m + 1],
                                     scale=g1[:, m:m + 1])
            for m in range(2):
                ps = ps_oth.tile([128, 512], F32, name="oth", tag="oth")
                for k in range(4):
                    nc.tensor.matmul(ps[:, 0:512],
                                     lhsT=w2[k][:, m * 128:(m + 1) * 128],
                                     rhs=gpre[k][:, qsl],
                                     start=(k == 0), stop=(k == 3))
                xqf_t = scr.tile([128, 512], F32, name="xqf_t", tag="xqf_t")
                nc.sync.dma_start(out=xqf_t,
                                  in_=xqf[s][m * 128:(m + 1) * 128, qsl])
                o_sb = scr.tile([128, 512], F32, name="o_sb", tag="o_sb")
                nc.vector.scalar_tensor_tensor(
                    o_sb, ps[:, 0:512], b2[:, m:m + 1], xqf_t,
                    op0=AluOpType.add, op1=AluOpType.add)
                nc.sync.dma_start(out=o_dram[s][m * 128:(m + 1) * 128, qsl],
                                  in_=o_sb)

        # ================= emission order =================
        for _rep in range(reps):
            proj_q(0)
            proj_q(1)
            proj_kv(1)      # dir0 needs K1/V1
            proj_kv(0)
            attention(0)    # per-qb: drains + to_out + ffn chain pipelined
            attention(1)
            flush_pending()

    nc.compile()
    return nc


_CACHE = {}
LAST_RESULT = {}


def _get_program(nq, nk):
    key = (nq, nk)
    if key not in _CACHE:
        _CACHE[key] = build_program(nq, nk)
    return _CACHE[key]


def make_in_maps(x0, x1, Wqk, bqk, Wv, bv, Wo, bo, W1, b1, ln_g, ln_b, W2, b2):
    s = SCALE
    tb = lambda a: np.ascontiguousarray(a).astype(NPBF)
    tf = lambda a: np.ascontiguousarray(a, dtype=np.float32)
    fold = lambda v, m: tf(np.asarray(v, np.float32).reshape(m, 128).T)
    shared = {
        "wqk": tb(np.asarray(Wqk, np.float32) * s),
        "wv": tb(Wv), "wo": tb(Wo), "w1": tb(W1), "w2": tb(W2),
        "bqk": fold(np.asarray(bqk, np.float32) * s, 2),
        "bv": tb(np.asarray(bv, np.float32)[None, :]),
        "bo": fold(bo, 2), "b1": fold(b1, 4),
        "g1": fold(ln_g, 4), "lb": fold(ln_b, 4), "b2": fold(b2, 2),
    }
    nq = N // 4
    in_maps = []
    for c in range(8):
        b, r = c // 4, c % 4
        xs = [np.asarray(x0[b], np.float32), np.asarray(x1[b], np.float32)]
        m = dict(shared)
        for si in range(2):
            xT = np.ascontiguousarray(xs[si].T)        # [D, N]
            m[f"xk{si}"] = xT.astype(NPBF)
            qT = np.ascontiguousarray(xT[:, r * nq:(r + 1) * nq])
            m[f"xq{si}"] = qT.astype(NPBF)
            m[f"xqf{si}"] = qT
        in_maps.append(m)
    return in_maps


def kernel(x0, x1, Wqk, bqk, Wv, bv, Wo, bo, W1, b1, ln_g, ln_b, W2, b2,
           _trace=False):
    nc = _get_program(N // 4, N)
    in_maps = make_in_maps(x0, x1, Wqk, bqk, Wv, bv, Wo, bo, W1, b1,
                           ln_g, ln_b, W2, b2)
    # The axon-tunneled NRT occasionally reports a transient
    # NRT_EXEC_UNIT_UNRECOVERABLE; the device recovers, so retry.
    last_err = None
    for attempt in range(3):
        try:
            res = run_bass_kernel_spmd(nc, in_maps, core_ids=list(range(8)),
                                       trace=_trace)
            break
        except Exception as e:  # noqa: BLE001
            last_err = e
            if "UNRECOVERABLE" not in str(e) and "UNAVAILABLE" not in str(e):
                raise
            import time as _time
            _time.sleep(10)
    else:
        raise last_err
    LAST_RESULT["res"] = res
    nq = N // 4
    out0 = np.empty((B, N, D), np.float32)
    out1 = np.empty((B, N, D), np.float32)
    for c in range(8):
        b, r = c // 4, c % 4
        out0[b, r * nq:(r + 1) * nq] = res.results[c]["o0"].T
        out1[b, r * nq:(r + 1) * nq] = res.results[c]["o1"].T
    return out0, out1

